# revision 13
# baseline (speedup 1.0000x reference)
"""Trainium2 Bass kernel for nn_ExtractionLayer.

metric[b,v,f] = sum_p amp[b,f,p] * exp(-c*(vol[v]*filt[f] - q[b,p])^2)
  amp = softmax_p(logits[b,f,p]),  c = 0.5/(sigma+0.001)^2

Sharding: data-parallel over batch B=32 -> 4 b's per core on 8 cores.

Per-core algorithm (2 "sets", each set = 2 b's = 128 (b,p) partition pairs):
  PE pass 1 : S'[(b,p),(f,v)] = x^2 - 2qx via a K=9 bf16 matmul.
              bf16 is 4x faster than fp32 on the PE; fp32-level accuracy
              comes from 3-way hi/mid/lo bf16 splits of x^2, x and -2q
              (9 rows = 3 a-rows + {b_h*3, b_m*2, b_l*1} cross terms).
              Even/odd 512-col chunks use PE row-groups 0/32 and run
              concurrently. The q^2 term folds into the ACT bias.
  ACT pass  : E = exp(-c*S' - c*q^2)  PSUM->SBUF fp16, FD=1536 groups,
              double-buffered PSUM -- this ~55us exp pass is the floor.
  PE pass 2 : per (f, v-half): lhsT = E-slice (128,128) stationary,
              rhs = block-diag softmax weight pair (128,2) moving ->
              psum out (128 v, 2 b) -- partition-dense output.
  drain     : DVE copy psum->SBUF, DMA -> out[s,v,f,b']; host -> [b,v,f]

X rows are built on device (DVE splits of fil*vol outer product),
bounced through DRAM, and streamed into 4 column-piece tiles so PE-1
starts as soon as the first piece lands.
"""

import sys

for _p in ("/opt/trn_rl_repo", "/root/.axon_site/_ro/trn_rl_repo"):
    if _p not in sys.path:
        sys.path.append(_p)

import numpy as np

B, V, F, P = 32, 256, 128, 64
NCORES = 8
B_LOC = B // NCORES          # 4 batches per core
NSETS = B_LOC // 2           # 2 sets of (2 b's x 64 p) = 128 partitions
NVF = V * F                  # 32768 (f-major: i = f*V + v)
GROUP = 1536                 # ACT free dim (3 PSUM banks); last group ragged
# per-set group list: (start_col, n_cols), n_cols multiple of V
GROUPS = []
_c0 = 0
while _c0 < NVF:
    GROUPS.append((_c0, min(GROUP, NVF - _c0)))
    _c0 += GROUP

_cache: dict = {}


def _build(minus_c):
    import concourse.tile as tile
    from concourse import bacc, mybir

    fp32 = mybir.dt.float32
    fp16 = mybir.dt.float16
    bf16 = mybir.dt.bfloat16
    AF = mybir.ActivationFunctionType
    OP = mybir.AluOpType
    import concourse.bass as bass

    nc = bacc.Bacc("TRN2", target_bir_lowering=False, debug=False,
                   num_devices=NCORES)

    d_q = nc.dram_tensor("q", [B_LOC * P], fp32, kind="ExternalInput")
    d_lf = nc.dram_tensor("lf", [F, B_LOC, P], fp32, kind="ExternalInput")
    d_lt = nc.dram_tensor("lt", [B_LOC * P, F], fp32, kind="ExternalInput")
    d_vol = nc.dram_tensor("vol", [128, V], fp32, kind="ExternalInput")
    d_fil = nc.dram_tensor("fil", [F], fp32, kind="ExternalInput")
    d_sig = nc.dram_tensor("sig", [1], fp32, kind="ExternalInput")
    # out[s, v, f, b'] -> contiguous DMA per (set, v-half); host interleaves
    d_out = nc.dram_tensor("out", [NSETS, V, F, 2], fp32, kind="ExternalOutput")
    d_zb = nc.dram_tensor("zb", [B_LOC * F], fp32)  # Zinv bounce, [b][f]
    # split bounces: piece p depends only on its f-quarter q=p
    d_xb_b = [nc.dram_tensor(f"xbb{q}", [32, 3 * V], bf16) for q in range(4)]
    d_xb_a = [nc.dram_tensor(f"xba{q}", [32, 3 * V], bf16) for q in range(4)]

    with tile.TileContext(nc) as tc:
        with (
            tc.tile_pool(name="const", bufs=1) as cp,
            tc.tile_pool(name="ering", bufs=10) as ep,
            tc.tile_pool(name="psS", bufs=2, space=bass.MemorySpace.PSUM) as psS,
            tc.tile_pool(name="psO", bufs=2, space=bass.MemorySpace.PSUM) as psO,
        ):
            # round-robin DMA issue over engines that sit mostly idle
            dmaeng = [nc.sync, nc.gpsimd]
            dmactr = [0]

            def dma(dst, src):
                e = dmaeng[dmactr[0] % len(dmaeng)]
                dmactr[0] += 1
                e.dma_start(dst, src)

            # setup-only rotation may also use the ACT engine's DMA port
            dmaeng3 = [nc.sync, nc.gpsimd, nc.scalar]

            def dma3(dst, src):
                e = dmaeng3[dmactr[0] % len(dmaeng3)]
                dmactr[0] += 1
                e.dma_start(dst, src)

            # ---- queue plan ----
            # sync   : q/fil/vol loads, packed->DRAM bounce, X bands 0-1
            # scalar : softmax exps, Wq DMAs, X bands 2-3, then main exps
            # gpsimd : lf/lt loads, zb/zr chain, steady-state output DMAs
            # vector : x/splits, Z/Zinv, srow, Wamp, drains
            volr = cp.tile([128, V], fp32, tag="volr")
            nc.sync.dma_start(volr[:, :], d_vol.ap())
            filc = cp.tile([128, 1], fp32, tag="filc")
            nc.sync.dma_start(filc[:, :], d_fil.ap().rearrange("(f o) -> f o", o=1))
            q_row = cp.tile([1, B_LOC * P], fp32, tag="q_row")
            nc.sync.dma_start(q_row[:, :], d_q.ap())
            qcol = cp.tile([128, NSETS], fp32, tag="qcol")
            for s in range(NSETS):
                nc.gpsimd.dma_start(
                    qcol[:, s:s + 1],
                    d_q.ap().rearrange("(k o) -> k o", o=1)[s * 128:(s + 1) * 128, :])
            lf_sb = cp.tile([128, B_LOC, P], fp32, tag="lf_sb")
            nc.gpsimd.dma_start(lf_sb[:, :, :], d_lf.ap())
            lt_sb = []
            for s in range(NSETS):
                t = cp.tile([128, F], fp32, tag=f"lt{s}", name=f"lt{s}")
                nc.gpsimd.dma_start(t[:, :], d_lt.ap()[s * 128:(s + 1) * 128, :])
                lt_sb.append(t)

            # softmax exps early on the ACT queue
            el_f = cp.tile([128, B_LOC, P], fp32, tag="el_f")
            nc.scalar.activation(el_f[:, :, :], lf_sb[:, :, :], AF.Exp)
            elt = []
            for s in range(NSETS):
                e = cp.tile([128, F], fp32, tag=f"elt{s}", name=f"elt{s}")
                nc.scalar.activation(e[:, :], lt_sb[s][:, :], AF.Exp)
                elt.append(e)

            # ---- x = fil*vol and bf16 splits (DVE), packed col-blocks ----
            # packed[:, blk*V:(blk+1)*V] blocks: a_h a_m a_l b_h b_m b_l 1
            x_ft = cp.tile([128, V], fp32, tag="x_ft")
            nc.vector.tensor_scalar(x_ft[:, :], volr[:, :], filc[:, 0:1], None,
                                    op0=OP.mult)
            xsq_ft = cp.tile([128, V], fp32, tag="xsq_ft")
            nc.vector.tensor_tensor(xsq_ft[:, :], x_ft[:, :], x_ft[:, :], OP.mult)
            packed = cp.tile([128, 6 * V], bf16, tag="packed")
            # blocks 0-2: b=x splits (bounced first), 3-5: a=x^2 splits

            def split3(val32, blk, eng):
                """bf16-split val32 into packed blocks blk, blk+1, blk+2."""
                r1 = cp.tile([128, V], fp32, tag=f"r1_{blk}", name=f"r1_{blk}")
                r2 = cp.tile([128, V], fp32, tag=f"r2_{blk}", name=f"r2_{blk}")
                h = packed[:, blk * V:(blk + 1) * V]
                m = packed[:, (blk + 1) * V:(blk + 2) * V]
                l = packed[:, (blk + 2) * V:(blk + 3) * V]
                eng.tensor_copy(h, val32[:, :])
                eng.tensor_tensor(r1[:, :], val32[:, :], h, OP.subtract)
                eng.tensor_copy(m, r1[:, :])
                eng.tensor_tensor(r2[:, :], r1[:, :], m, OP.subtract)
                eng.tensor_copy(l, r2[:, :])

            split3(x_ft, 0, nc.vector)
            for q in range(4):
                bap = bass.AP(tensor=d_xb_b[q], offset=0,
                              ap=[[V, 32], [32 * V, 3], [1, V]])
                (nc.sync if q % 2 else nc.gpsimd).dma_start(
                    bap, packed[32 * q:32 * (q + 1), 0:3 * V])
            split3(xsq_ft, 3, nc.vector)
            for q in range(4):
                aap = bass.AP(tensor=d_xb_a[q], offset=0,
                              ap=[[V, 32], [32 * V, 3], [1, V]])
                (nc.gpsimd if q % 2 else nc.sync).dma_start(
                    aap, packed[32 * q:32 * (q + 1), 3 * V:6 * V])

            # exp bias per set: -c * q^2  (folds the q^2 matmul rows away)
            ebias = cp.tile([128, NSETS], fp32, tag="ebias")
            nc.vector.tensor_tensor(ebias[:, :], qcol[:, :], qcol[:, :], OP.mult)
            nc.vector.tensor_scalar_mul(ebias[:, :], ebias[:, :], float(minus_c))
            # ---- X: two row-group bands (PE rows 0-8 / 32-40) so even/odd
            # chunks run as concurrent matmuls in different 32-row groups.
            # 4 column-piece tiles stream in while PE consumes them.
            # Piece tile rows 0-8 = band 0 (even chunks), 32-40 = band 1.
            # uneven pieces: small first pieces -> earliest PE start
            PJ = [(0, 4), (4, 4), (8, 8), (16, 8), (24, 8)]  # (j0, nj)
            PB = [0, 2048, 4096, 8192, 12288, 16384]          # col bounds
            NP = 8  # j-dim stride unit in source AP (8 j's per f-quarter)
            Xp = [cp.tile([41, nj * 512], bf16, tag=f"Xp{p}", name=f"Xp{p}")
                  for p, (j0, nj) in enumerate(PJ)]
            # (dst_row0, n_rows, src_block, src_row_stride_in_blocks, bounce)
            rowgrps = [(3, 3, 0, 0, d_xb_b), (6, 2, 1, 0, d_xb_b),
                       (8, 1, 2, 0, d_xb_b), (0, 3, 0, 1, d_xb_a)]
            xeng = [nc.sync, nc.gpsimd]
            xi = 0

            def load_piece(p):
                # piece p covers within-band chunks j in [j0, j0+nj); band b
                # chunk j holds f = 4j + 2b + fi (f-major, 512-col chunks);
                # bounce layout [blk][f][v] makes each chunk a 512-run
                nonlocal xi
                j0, nj = PJ[p]
                q = j0 // 8                           # source f-quarter
                for band in range(2):
                    for (r0, nr, blk, rstr, dxb) in rowgrps:
                        srcap = bass.AP(
                            tensor=dxb[q],
                            offset=blk * 32 * V + (2 * band) * V
                                   + (j0 % 8) * 4 * V,
                            ap=[[rstr * 32 * V, nr], [4 * V, nj],
                                [1, 2 * V]],
                        )
                        dstap = Xp[p][32 * band + r0:32 * band + r0 + nr, :]
                        dstap = dstap.rearrange("r (j w) -> r j w", j=nj)
                        xeng[xi % 2].dma_start(dstap, srcap)
                        xi += 1

            load_piece(0)
            load_piece(1)
            load_piece(2)

            # ---- W_q rows (DVE srow chains), DMAs on scalar ----
            Wq = []
            wt32 = cp.tile([1, 128], fp32, tag="wt32")
            res1 = cp.tile([1, 128], fp32, tag="res1")
            res2 = cp.tile([1, 128], fp32, tag="res2")

            def split3_row(val32, srow, cols):
                """bf16-split val32 (1,128) into 128-col slices of srow.
                Runs on gpsimd: tiny ops, keeps DVE free for the X splits."""
                cur = val32
                for i, cidx in enumerate(cols):
                    hb = srow[0:1, cidx * 128:(cidx + 1) * 128]
                    nc.vector.tensor_copy(hb, cur[:, :])
                    if i < len(cols) - 1:
                        dst = res1 if cur is not res1 else res2
                        nc.vector.tensor_tensor(dst[:, :], cur[:, :], hb,
                                                OP.subtract)
                        cur = dst

            for s in range(NSETS):
                srow = cp.tile([1, 9 * 128], bf16, tag=f"srow{s}",
                               name=f"srow{s}")
                w = cp.tile([41, 128], bf16, tag=f"Wq{s}", name=f"Wq{s}")
                qs = q_row[0:1, s * 128:(s + 1) * 128]
                nc.vector.memset(srow[0:1, 0:3 * 128], 1.0)
                nc.vector.tensor_scalar_mul(wt32[:, :], qs, -2.0)
                split3_row(wt32, srow, [3, 4, 5])   # w_h w_m w_l
                nc.vector.tensor_copy(srow[0:1, 6 * 128:7 * 128],
                                      srow[0:1, 3 * 128:4 * 128])  # w_h
                nc.vector.tensor_copy(srow[0:1, 7 * 128:8 * 128],
                                      srow[0:1, 4 * 128:5 * 128])  # w_m
                nc.vector.tensor_copy(srow[0:1, 8 * 128:9 * 128],
                                      srow[0:1, 3 * 128:4 * 128])  # w_h
                nc.sync.dma_start(w[0:9, :], srow[0:1, :])
                nc.gpsimd.dma_start(w[32:41, :], srow[0:1, :])
                Wq.append(w)

            # ---- softmax denominators (DVE) + zb/zr chain (gpsimd) ----
            Z = cp.tile([128, B_LOC], fp32, tag="Z")
            nc.vector.tensor_reduce(Z[:, :], el_f[:, :, :], mybir.AxisListType.X,
                                    OP.add)
            Zinv = cp.tile([128, B_LOC], fp32, tag="Zinv")
            nc.vector.reciprocal(Zinv[:, :], Z[:, :])
            nc.gpsimd.dma_start(
                bass.AP(tensor=d_zb, offset=0, ap=[[1, 128], [128, B_LOC]]),
                Zinv[:, :])
            zrs = []
            for s in range(NSETS):
                zr = cp.tile([128, F], fp32, tag=f"zr{s}", name=f"zr{s}")
                for h in range(2):
                    nc.gpsimd.dma_start(
                        zr[h * 64:(h + 1) * 64, :],
                        bass.AP(tensor=d_zb, offset=(2 * s + h) * F,
                                ap=[[0, 64], [1, F]]))
                zrs.append(zr)

            load_piece(3)
            load_piece(4)

            # ---- W_amp: block-diag fp16 softmax weights (DVE) ----
            # W_amp[k=(b,p), 2f+h] = amp[b,f,p] for k//64==h else 0
            Wamp = []
            for s in range(NSETS):
                w = cp.tile([128, 2 * F], fp16, tag=f"Wamp{s}", name=f"Wamp{s}")
                nc.vector.memset(w[:, :], 0.0)
                for h in range(2):
                    nc.vector.tensor_tensor(
                        w[h * 64:(h + 1) * 64, h:2 * F:2],
                        elt[s][h * 64:(h + 1) * 64, :],
                        zrs[s][h * 64:(h + 1) * 64, :],
                        OP.mult,
                    )
                Wamp.append(w)

            # ---- main pipeline ----
            for s in range(NSETS):
                # psum out accumulator per set, cols [vh*256 + 2*f + b']
                sO = psO.tile([128, 2 * 2 * F], fp32, tag="O", name="sO")
                for (g0, gc) in GROUPS:
                    sS = psS.tile([128, GROUP], fp32, tag="S", name="sS")
                    for ci in range(gc // 512):
                        c = (g0 + ci * 512) // 512        # global chunk
                        band, bloc = c % 2, (c // 2) * 512
                        p = next(k for k in range(5)
                                 if PB[k] <= bloc < PB[k + 1])
                        loc = bloc - PB[p]
                        nc.tensor.matmul(
                            sS[:, ci * 512:(ci + 1) * 512],
                            Wq[s][32 * band:32 * band + 9, :],
                            Xp[p][32 * band:32 * band + 9, loc:loc + 512],
                            start=True, stop=True,
                            tile_position=(32 * band, 0),
                        )
                    E = ep.tile([128, GROUP], fp16, tag="E", name="E")
                    nc.scalar.activation(E[:, 0:gc], sS[:, 0:gc], AF.Exp,
                                         scale=float(minus_c),
                                         bias=ebias[:, s:s + 1])
                    for fr in range(gc // V):
                        f = g0 // V + fr                       # global f
                        for vh in range(2):
                            base = vh * 2 * F
                            nc.tensor.matmul(
                                sO[:, base + 2 * f:base + 2 * f + 2],
                                E[:, fr * V + vh * 128:fr * V + vh * 128 + 128],
                                Wamp[s][:, 2 * f:2 * f + 2],
                                start=True, stop=True,
                            )
                # drain psum -> sbuf -> DRAM out[s, v, f, b'] (contiguous)
                for vh in range(2):
                    ob = cp.tile([128, 2 * F], fp32, tag=f"ob{vh}",
                                 name=f"ob{vh}")
                    base = vh * 2 * F
                    nc.vector.tensor_copy(ob[:, :], sO[:, base:base + 2 * F])
                    nc.gpsimd.dma_start(
                        d_out.ap()[s:s + 1, vh * 128:(vh + 1) * 128, :, :],
                        ob[:, :],
                    )

    nc.compile()
    return nc


def _get_nc(minus_c):
    key = float(minus_c)
    if key not in _cache:
        _cache[key] = _build(key)
    return _cache[key]


def kernel(q2_obs_scaled, amplitude_logits, volumes, filters, sigma,
           _trace=False, _tmpdir=None):
    from concourse.bass_utils import run_bass_kernel_spmd

    minus_c = -0.5 / (float(np.asarray(sigma).reshape(())) + 0.001) ** 2
    nc = _get_nc(minus_c)

    q = np.ascontiguousarray(np.asarray(q2_obs_scaled, dtype=np.float32))
    lg = np.asarray(amplitude_logits, dtype=np.float32).reshape(B, F, P)
    vol = np.ascontiguousarray(np.asarray(volumes, dtype=np.float32).reshape(V))
    fil = np.ascontiguousarray(np.asarray(filters, dtype=np.float32).reshape(F))
    sig = np.asarray(sigma, dtype=np.float32).reshape(1)

    in_maps = []
    for i in range(NCORES):
        bsl = slice(i * B_LOC, (i + 1) * B_LOC)
        lgc = lg[bsl]                                    # (B_LOC, F, P)
        in_maps.append({
            "q": np.ascontiguousarray(q[bsl].reshape(B_LOC * P)),
            "lf": np.ascontiguousarray(lgc.transpose(1, 0, 2)),   # (F,B_LOC,P)
            "lt": np.ascontiguousarray(
                lgc.transpose(0, 2, 1).reshape(B_LOC * P, F)),    # ((b,p),F)
            "vol": np.broadcast_to(vol, (128, V)).copy(),
            "fil": fil,
            "sig": sig,
        })

    kw = {}
    if _trace:
        kw = {"trace": True, "tmpdir": _tmpdir}
    res = run_bass_kernel_spmd(nc, in_maps, core_ids=list(range(NCORES)), **kw)

    out = np.empty((B, V, F), dtype=np.float32)
    for i in range(NCORES):
        oc = res.results[i]["out"]                       # (NSETS, V, F, 2)
        for s in range(NSETS):
            for h in range(2):
                out[i * B_LOC + 2 * s + h] = oc[s, :, :, h]
    if _trace:
        return out, res
    return out

